# revision 11
# baseline (speedup 1.0000x reference)
"""Cross-attention kernel for 8 Trainium2 NeuronCores.

Problem (hardcoded): x [4,4096,512], context [4,1024,768], 8 heads x 64,
inner 512. out = softmax((x@Wq)(ctx@Wk)^T / 8) @ (ctx@Wv) @ Wo + bo.

Sharding: 8 cores = 4 batches x 2 head-groups (4 heads each).
Core c handles batch b=c//2, heads [4g, 4g+4) with g=c%2:
  - Wq/Wk/Wv column-sliced, Wo row-sliced (tensor parallel over heads)
  - each core emits a partial [4096, 512]; host sums the two head-group
    partials per batch and adds bo (the only host math; everything else
    runs on device).

Device-side layout choices:
  - host pre-transposes x/context so all projections contract naturally
    (feature dim on partitions); no on-device transposes at all.
  - qT/kT hold head pairs stacked on partitions (rows 0-63 = even head,
    64-127 = odd head) so the K=64 score matmuls for the two heads occupy
    disjoint PE row-groups and run concurrently (full array).
  - scores are built transposed [keys, q] so softmax exp feeds the AV
    matmul directly as the moving operand; v carries an extra ones column
    so the softmax denominator falls out of the AV matmul for free.
  - exp runs in [128, 2048] ACT calls out of a 4-bank PSUM tile.
"""

import os
import sys

for _p in ("/opt/trn_rl_repo", "/root/.axon_site/_ro/trn_rl_repo"):
    if os.path.isdir(_p) and _p not in sys.path:
        sys.path.append(_p)

import numpy as np

import concourse.bass as bass  # noqa: E402
import concourse.mybir as mybir  # noqa: E402
import concourse.tile as tile  # noqa: E402
from concourse import bacc  # noqa: E402
from concourse import bass_utils  # noqa: E402

P = 128
B = 4
NQ = 4096  # queries per batch
DX = 512  # x feature dim (4 chunks of 128)
NC = 1024  # context length (8 key chunks of 128)
DC = 768  # context feature dim (6 chunks of 128)
DH = 64  # head dim
HPC = 4  # heads per core
COLS = HPC * DH  # 256 = per-core slice of the inner dim
DOUT = 512  # output dim

DXC = DX // P  # 4
DCC = DC // P  # 6
KC = NC // P  # 8
NQT = NQ // 512  # 8 query tiles of 512

F32 = mybir.dt.float32
F32R = mybir.dt.float32r
EXP = mybir.ActivationFunctionType.Exp
SCALE = DH**-0.5  # 0.125, folded into the exp activation's scale


def _r(ap):
    return ap.bitcast(F32R)


def _emit(tc, nc, xT, ctxT, wq, wk, wv, wo, ones, out):
    with (
        tc.tile_pool(name="consts", bufs=1) as consts,
        tc.tile_pool(name="xstream", bufs=3) as xstream,
        tc.tile_pool(name="etile", bufs=3) as etile,
        tc.tile_pool(name="norm", bufs=2) as norm,
        tc.tile_pool(name="dscr", bufs=4, space="DRAM") as dscr,
    ):
        # ---- weights + context into SBUF (feature dim on partitions) ----
        wq_sb = consts.tile([P, DXC, COLS], F32R, tag="wq", name="wq_sb")
        wk_sb = consts.tile([P, DCC, COLS], F32R, tag="wk", name="wk_sb")
        wv_sb = consts.tile([P, DCC, COLS], F32R, tag="wv", name="wv_sb")
        wo_sb = consts.tile([P, 2, DOUT], F32R, tag="wo", name="wo_sb")
        ctxT_sb = consts.tile([P, DCC, NC], F32R, tag="ctxT", name="ctxT_sb")
        nc.sync.dma_start(wq_sb[:], wq.rearrange("(c p) n -> p c n", p=P).bitcast(F32R))
        nc.sync.dma_start(wk_sb[:], wk.rearrange("(c p) n -> p c n", p=P).bitcast(F32R))
        nc.sync.dma_start(wv_sb[:], wv.rearrange("(c p) n -> p c n", p=P).bitcast(F32R))
        nc.sync.dma_start(wo_sb[:], wo.rearrange("(c p) n -> p c n", p=P).bitcast(F32R))
        nc.sync.dma_start(ctxT_sb[:], ctxT.rearrange("(c p) n -> p c n", p=P).bitcast(F32R))

        ps_proj_cm = tc.tile_pool(name="ps_proj", bufs=2, space="PSUM")
        ps_proj = ps_proj_cm.__enter__()

        # ---- K^T projection: kT[pair][2*64 head dims, 1024 keys] ----
        kT_sb = [consts.tile([P, NC], F32R, tag=f"kT{p}", name=f"kT{p}") for p in range(2)]
        for p in range(2):
            for ks in range(2):
                acc = ps_proj.tile([P, 512], F32, tag="proj", name="kproj_acc")
                for ch in range(DCC):
                    nc.tensor.matmul(
                        acc[:],
                        wk_sb[:, ch, p * P : (p + 1) * P],
                        ctxT_sb[:, ch, ks * 512 : (ks + 1) * 512],
                        start=(ch == 0),
                        stop=(ch == DCC - 1),
                    )
                nc.vector.tensor_copy(kT_sb[p][:, ks * 512 : (ks + 1) * 512], acc[:])

        # ---- V projection, keys on partitions, + ones column ----
        # v_sb[:, kc, h, 0:64] = V for head h, key chunk kc; [..., 64] = 1.0
        v_sb = consts.tile([P, KC, HPC, DH + 1], F32R, tag="v", name="v_sb")
        # memset cannot write f32r; broadcast a DRAM 1.0 into the ones column
        nc.sync.dma_start(
            v_sb[:, :, :, DH : DH + 1].rearrange("p a b o -> p (a b o)"),
            ones.to_broadcast((P, KC * HPC)).bitcast(F32R),
        )
        for kc in range(KC):
            acc = ps_proj.tile([P, 512], F32, tag="proj", name="vproj_acc")
            for ch in range(DCC):
                nc.tensor.matmul(
                    acc[:, 0:COLS],
                    ctxT_sb[:, ch, kc * P : (kc + 1) * P],
                    wv_sb[:, ch, :],
                    start=(ch == 0),
                    stop=(ch == DCC - 1),
                )
            nc.vector.tensor_copy(
                v_sb[:, kc, :, 0:DH], acc[:, 0:COLS].rearrange("p (h d) -> p h d", d=DH)
            )

        # ---- Q^T projection: per (pair, 512-query slice) tiles ----
        qT_sb = {}
        for qs in range(NQT):
            xt = xstream.tile([P, DXC, 512], F32R, tag="xt", name="xt")
            nc.sync.dma_start(
                xt[:],
                xT.rearrange("(c p) q -> p c q", p=P)[:, :, qs * 512 : (qs + 1) * 512].bitcast(F32R),
            )
            for p in range(2):
                acc = ps_proj.tile([P, 512], F32, tag="proj", name="qproj_acc")
                for ch in range(DXC):
                    nc.tensor.matmul(
                        acc[:],
                        wq_sb[:, ch, p * P : (p + 1) * P],
                        xt[:, ch, :],
                        start=(ch == 0),
                        stop=(ch == DXC - 1),
                    )
                qt_t = consts.tile([P, 512], F32R, tag=f"qT{p}_{qs}", name=f"qT{p}_{qs}")
                qT_sb[(p, qs)] = qt_t
                nc.vector.tensor_copy(qt_t[:], acc[:])

        ps_proj_cm.__exit__(None, None, None)
        ps_scores_cm = tc.tile_pool(name="ps_scores", bufs=1, space="PSUM")
        ps_scores = ps_scores_cm.__enter__()
        ps_attn_cm = tc.tile_pool(name="ps_attn", bufs=2, space="PSUM")
        ps_attn = ps_attn_cm.__enter__()
        ps_out_cm = tc.tile_pool(name="ps_out", bufs=2, space="PSUM")
        ps_out = ps_out_cm.__enter__()

        # ---- attention + output projection, per 512-query tile ----
        for qt in range(NQT):
            attnT = {}
            for p in range(2):
                at_t = consts.tile([P, 512], F32R, tag=f"attnT{p}_{qt}", name=f"attnT{p}_{qt}")
                attnT[p] = at_t
                qt_t = qT_sb[(p, qt)]
                # attn accumulators for the two heads of this pair
                accs = [ps_attn.tile([DH + 1, 512], F32, tag="attnT", name="attn_acc") for _ in range(2)]
                for g in range(4):  # groups of 2 key chunks
                    sc = ps_scores.tile([P, 2, 2, 512], F32, tag="scores", name="scores_ps")
                    for jj in range(2):
                        kc = g * 2 + jj
                        for j in range(2):
                            nc.tensor.matmul(
                                sc[:, jj, j, :],
                                kT_sb[p][j * DH : (j + 1) * DH, kc * P : (kc + 1) * P],
                                qt_t[j * DH : (j + 1) * DH, :],
                                start=True,
                                stop=True,
                            )
                    ex = etile.tile([P, 2, 2, 512], F32R, tag="exp", name="exp_sb")
                    nc.scalar.activation(ex[:], sc[:], EXP, scale=SCALE)
                    for j in range(2):
                        for jj in range(2):
                            kc = g * 2 + jj
                            nc.tensor.matmul(
                                accs[j][:],
                                v_sb[:, kc, 2 * p + j, :],
                                ex[:, jj, j, :],
                                start=(kc == 0),
                                stop=(kc == KC - 1),
                            )
                # normalize: row 64 of each accumulator is the softmax denom
                for j in range(2):
                    # denom row lives on partition 64; engines cannot shift
                    # partitions, so reciprocal lands on partition 64 of an
                    # SBUF tile and DMA broadcasts it down to partitions 0-63
                    den = norm.tile([DH + 1, 512], F32, tag="denom", name="den_t")
                    nc.vector.reciprocal(
                        den[DH : DH + 1, :], accs[j][DH : DH + 1, :]
                    )
                    # SBUF sources cannot partition-broadcast in DMA; bounce
                    # the reciprocal row through DRAM (DRAM sources can)
                    dden = dscr.tile([1, 512], F32, tag="dden", name="dden_t")
                    nc.sync.dma_start(dden[:], den[DH : DH + 1, :])
                    rec = norm.tile([DH, 512], F32, tag="recip", name="recip_t")
                    nc.sync.dma_start(rec[:], dden[:].to_broadcast((DH, 512)))
                    if j == 0:
                        nc.vector.tensor_mul(at_t[0:DH, :], accs[j][0:DH, :], rec[:])
                    else:
                        tmp = norm.tile([DH, 512], F32R, tag="normtmp", name="normtmp_t")
                        nc.vector.tensor_mul(tmp[:], accs[j][0:DH, :], rec[:])
                        # engines cannot shift partitions; DMA moves the odd
                        # head's rows into partitions 64-127
                        nc.sync.dma_start(at_t[DH:P, :], tmp[:])
            for sub in range(4):
                o = ps_out.tile([P, DOUT], F32, tag="oproj", name="oproj_acc")
                for p in range(2):
                    nc.tensor.matmul(
                        o[:],
                        attnT[p][:, sub * P : (sub + 1) * P],
                        wo_sb[:, p, :],
                        start=(p == 0),
                        stop=(p == 1),
                    )
                ostage = norm.tile([P, DOUT], F32, tag="ostage", name="ostage_t")
                nc.vector.tensor_copy(ostage[:], o[:])
                row = qt * 512 + sub * P
                nc.sync.dma_start(out[row : row + P, :], ostage[:])
        ps_out_cm.__exit__(None, None, None)
        ps_attn_cm.__exit__(None, None, None)
        ps_scores_cm.__exit__(None, None, None)


def _build():
    nc = bacc.Bacc(
        "TRN2", target_bir_lowering=False, debug=False, enable_asserts=False
    )
    xT = nc.dram_tensor("xT", [DX, NQ], F32, kind="ExternalInput").ap()
    ctxT = nc.dram_tensor("ctxT", [DC, NC], F32, kind="ExternalInput").ap()
    wq = nc.dram_tensor("wq", [DX, COLS], F32, kind="ExternalInput").ap()
    wk = nc.dram_tensor("wk", [DC, COLS], F32, kind="ExternalInput").ap()
    wv = nc.dram_tensor("wv", [DC, COLS], F32, kind="ExternalInput").ap()
    wo = nc.dram_tensor("wo", [COLS, DOUT], F32, kind="ExternalInput").ap()
    ones = nc.dram_tensor("ones", [1, KC * HPC], F32, kind="ExternalInput").ap()
    out = nc.dram_tensor("out", [NQ, DOUT], F32, kind="ExternalOutput").ap()
    with tile.TileContext(nc) as tc:
        _emit(tc, nc, xT, ctxT, wq, wk, wv, wo, ones, out)
    nc.compile()
    return nc


_NC = None


def _get_nc():
    global _NC
    if _NC is None:
        _NC = _build()
    return _NC


def _in_maps(x, context, Wq, Wk, Wv, Wo):
    maps = []
    for c in range(8):
        b, g = c // 2, c % 2
        cs = slice(g * COLS, (g + 1) * COLS)
        maps.append(
            {
                "xT": np.ascontiguousarray(x[b].T),
                "ctxT": np.ascontiguousarray(context[b].T),
                "wq": np.ascontiguousarray(Wq[:, cs]),
                "wk": np.ascontiguousarray(Wk[:, cs]),
                "wv": np.ascontiguousarray(Wv[:, cs]),
                "wo": np.ascontiguousarray(Wo[cs, :]),
                "ones": np.ones((1, KC * HPC), np.float32),
            }
        )
    return maps


def _execute(in_maps, **kw):
    return bass_utils.run_bass_kernel_spmd(
        _get_nc(), in_maps, core_ids=list(range(8)), **kw
    )


def kernel(x, context, Wq, Wk, Wv, Wo, bo):
    x = np.asarray(x, np.float32)
    context = np.asarray(context, np.float32)
    Wq = np.asarray(Wq, np.float32)
    Wk = np.asarray(Wk, np.float32)
    Wv = np.asarray(Wv, np.float32)
    Wo = np.asarray(Wo, np.float32)
    bo = np.asarray(bo, np.float32)
    res = _execute(_in_maps(x, context, Wq, Wk, Wv, Wo))
    parts = [r["out"] for r in res.results]
    out = np.empty((B, NQ, DOUT), np.float32)
    for b in range(B):
        out[b] = parts[2 * b] + parts[2 * b + 1] + bo[None, :]
    return out


# revision 15
# speedup vs baseline: 1.1482x; 1.1482x over previous
"""Cross-attention kernel for 8 Trainium2 NeuronCores.

Problem (hardcoded): x [4,4096,512], context [4,1024,768], 8 heads x 64,
inner 512. out = softmax((x@Wq)(ctx@Wk)^T / 8) @ (ctx@Wv) @ Wo + bo.

Sharding: 8 cores = 4 batches x 2 head-groups (4 heads each).
Core c handles batch b=c//2, heads [4g, 4g+4) with g=c%2:
  - Wq/Wk/Wv column-sliced, Wo row-sliced (tensor parallel over heads)
  - each core emits a partial [4096, 512]; host sums the two head-group
    partials per batch and adds bo (the only host math; everything else
    runs on device).

Device-side layout choices:
  - host pre-transposes x/context so all projections contract naturally
    (feature dim on partitions); no on-device transposes at all.
  - qT/kT hold head pairs stacked on partitions (rows 0-63 = even head,
    64-127 = odd head) so the K=64 score matmuls for the two heads occupy
    disjoint PE row-groups and run concurrently (full array).
  - scores are built transposed [keys, q] so softmax exp feeds the AV
    matmul directly as the moving operand; v carries an extra ones column
    so the softmax denominator falls out of the AV matmul for free.
  - exp runs in [128, 2048] ACT calls out of a 4-bank PSUM tile.
"""

import os
import sys

for _p in ("/opt/trn_rl_repo", "/root/.axon_site/_ro/trn_rl_repo"):
    if os.path.isdir(_p) and _p not in sys.path:
        sys.path.append(_p)

import numpy as np

import concourse.bass as bass  # noqa: E402
import concourse.mybir as mybir  # noqa: E402
import concourse.tile as tile  # noqa: E402
from concourse import bacc  # noqa: E402
from concourse import bass_utils  # noqa: E402

P = 128
B = 4
NQ = 4096  # queries per batch
DX = 512  # x feature dim (4 chunks of 128)
NC = 1024  # context length (8 key chunks of 128)
DC = 768  # context feature dim (6 chunks of 128)
DH = 64  # head dim
HPC = 4  # heads per core
COLS = HPC * DH  # 256 = per-core slice of the inner dim
DOUT = 512  # output dim

DXC = DX // P  # 4
DCC = DC // P  # 6
KC = NC // P  # 8
NQT = NQ // 512  # 8 query tiles of 512

F32 = mybir.dt.float32
F32R = mybir.dt.float32r
EXP = mybir.ActivationFunctionType.Exp
SCALE = DH**-0.5  # 0.125, folded into the exp activation's scale


def _r(ap):
    return ap.bitcast(F32R)


def _emit(tc, nc, xT, ctxT, wq, wk, wv, wo, ones, out):
    with (
        tc.tile_pool(name="consts", bufs=1) as consts,
        tc.tile_pool(name="xstream", bufs=3) as xstream,
        tc.tile_pool(name="etile", bufs=4) as etile,
        tc.tile_pool(name="norm", bufs=2) as norm,
        tc.tile_pool(name="dscr", bufs=4, space="DRAM") as dscr,
    ):
        # ---- weights + context into SBUF (feature dim on partitions) ----
        wq_sb = consts.tile([P, DXC, COLS], F32R, tag="wq", name="wq_sb")
        wk_sb = consts.tile([P, DCC, COLS], F32R, tag="wk", name="wk_sb")
        wv_sb = consts.tile([P, DCC, COLS], F32R, tag="wv", name="wv_sb")
        wo_sb = consts.tile([P, 2, DOUT], F32R, tag="wo", name="wo_sb")
        ctx_pool_cm = tc.tile_pool(name="ctxpool", bufs=1)
        ctx_pool = ctx_pool_cm.__enter__()
        ctxT_sb = ctx_pool.tile([P, DCC, NC], F32R, tag="ctxT", name="ctxT_sb")
        nc.sync.dma_start(wq_sb[:], wq.rearrange("(c p) n -> p c n", p=P).bitcast(F32R))
        nc.sync.dma_start(wk_sb[:], wk.rearrange("(c p) n -> p c n", p=P).bitcast(F32R))
        nc.sync.dma_start(wv_sb[:], wv.rearrange("(c p) n -> p c n", p=P).bitcast(F32R))
        nc.sync.dma_start(wo_sb[:], wo.rearrange("(c p) n -> p c n", p=P).bitcast(F32R))
        nc.sync.dma_start(ctxT_sb[:], ctxT.rearrange("(c p) n -> p c n", p=P).bitcast(F32R))

        ps_proj_cm = tc.tile_pool(name="ps_proj", bufs=2, space="PSUM")
        ps_proj = ps_proj_cm.__enter__()

        # ---- K^T projection: kT[pair][2*64 head dims, 1024 keys] ----
        kT_sb = [consts.tile([P, NC], F32R, tag=f"kT{p}", name=f"kT{p}") for p in range(2)]
        for p in range(2):
            for ks in range(2):
                acc = ps_proj.tile([P, 512], F32, tag="proj", name="kproj_acc")
                for ch in range(DCC):
                    nc.tensor.matmul(
                        acc[:],
                        wk_sb[:, ch, p * P : (p + 1) * P],
                        ctxT_sb[:, ch, ks * 512 : (ks + 1) * 512],
                        start=(ch == 0),
                        stop=(ch == DCC - 1),
                    )
                nc.vector.tensor_copy(kT_sb[p][:, ks * 512 : (ks + 1) * 512], acc[:])

        # ---- V projection, keys on partitions, + ones column ----
        # v_sb[:, kc, h, 0:64] = V for head h, key chunk kc; [..., 64] = 1.0
        v_sb = consts.tile([P, KC, HPC, DH + 1], F32R, tag="v", name="v_sb")
        # memset cannot write f32r; broadcast a DRAM 1.0 into the ones column
        nc.sync.dma_start(
            v_sb[:, :, :, DH : DH + 1].rearrange("p a b o -> p (a b o)"),
            ones.to_broadcast((P, KC * HPC)).bitcast(F32R),
        )
        for kc in range(KC):
            acc = ps_proj.tile([P, 512], F32, tag="proj", name="vproj_acc")
            for ch in range(DCC):
                nc.tensor.matmul(
                    acc[:, 0:COLS],
                    ctxT_sb[:, ch, kc * P : (kc + 1) * P],
                    wv_sb[:, ch, :],
                    start=(ch == 0),
                    stop=(ch == DCC - 1),
                )
            nc.vector.tensor_copy(
                v_sb[:, kc, :, 0:DH], acc[:, 0:COLS].rearrange("p (h d) -> p h d", d=DH)
            )

        # ---- Q^T projection: per (pair, 512-query slice) tiles ----
        qT_sb = {}
        for qs in range(NQT):
            xt = xstream.tile([P, DXC, 512], F32R, tag="xt", name="xt")
            nc.sync.dma_start(
                xt[:],
                xT.rearrange("(c p) q -> p c q", p=P)[:, :, qs * 512 : (qs + 1) * 512].bitcast(F32R),
            )
            for p in range(2):
                acc = ps_proj.tile([P, 512], F32, tag="proj", name="qproj_acc")
                for ch in range(DXC):
                    nc.tensor.matmul(
                        acc[:],
                        wq_sb[:, ch, p * P : (p + 1) * P],
                        xt[:, ch, :],
                        start=(ch == 0),
                        stop=(ch == DXC - 1),
                    )
                qt_t = consts.tile([P, 512], F32R, tag=f"qT{p}_{qs}", name=f"qT{p}_{qs}")
                qT_sb[(p, qs)] = qt_t
                nc.vector.tensor_copy(qt_t[:], acc[:])

        ctx_pool_cm.__exit__(None, None, None)
        ps_proj_cm.__exit__(None, None, None)
        ps_scores_cm = tc.tile_pool(name="ps_scores", bufs=1, space="PSUM")
        ps_scores = ps_scores_cm.__enter__()
        ps_attn_cm = tc.tile_pool(name="ps_attn", bufs=3, space="PSUM")
        ps_attn = ps_attn_cm.__enter__()
        ps_out_cm = tc.tile_pool(name="ps_out", bufs=1, space="PSUM")
        ps_out = ps_out_cm.__enter__()

        # ---- attention + output projection, per 512-query tile ----
        for qt in range(NQT):
            attnT = {}
            for p in range(2):
                at_t = consts.tile([P, 512], F32R, tag=f"attnT{p}_{qt}", name=f"attnT{p}_{qt}")
                attnT[p] = at_t
                qt_t = qT_sb[(p, qt)]
                # attn accumulators for the two heads of this pair
                accs = [ps_attn.tile([DH + 1, 512], F32, tag="attnT", name="attn_acc") for _ in range(2)]
                for g in range(4):  # groups of 2 key chunks
                    sc = ps_scores.tile([P, 2, 2, 512], F32, tag="scores", name="scores_ps")
                    for jj in range(2):
                        kc = g * 2 + jj
                        for j in range(2):
                            nc.tensor.matmul(
                                sc[:, jj, j, :],
                                kT_sb[p][j * DH : (j + 1) * DH, kc * P : (kc + 1) * P],
                                qt_t[j * DH : (j + 1) * DH, :],
                                start=True,
                                stop=True,
                            )
                    ex = etile.tile([P, 2, 2, 512], F32R, tag="exp", name="exp_sb")
                    nc.scalar.activation(ex[:], sc[:], EXP, scale=SCALE)
                    for j in range(2):
                        for jj in range(2):
                            kc = g * 2 + jj
                            nc.tensor.matmul(
                                accs[j][:],
                                v_sb[:, kc, 2 * p + j, :],
                                ex[:, jj, j, :],
                                start=(kc == 0),
                                stop=(kc == KC - 1),
                            )
                # normalize: row 64 of each accumulator is the softmax denom.
                # InstReciprocal costs ~6.5ns per FREE element regardless of
                # partition count, so reshape both heads' denominators into a
                # [128, 8] tile via a DRAM bounce and reciprocal 1024 values
                # in one cheap op, then broadcast the results back down.
                dstage = norm.tile([DH + 1, 2, 512], F32, tag="denom", name="den_t")
                for j in range(2):
                    nc.vector.tensor_copy(
                        dstage[DH : DH + 1, j, :], accs[j][DH : DH + 1, :]
                    )
                dden = dscr.tile([1, 1024], F32, tag="dden", name="dden_t")
                nc.gpsimd.dma_start(dden[:], dstage[DH : DH + 1, :, :])
                rt = norm.tile([P, 8], F32, tag="rt", name="rt_t")
                nc.gpsimd.dma_start(rt[:], dden[0, :].rearrange("(p f) -> p f", p=P))
                nc.vector.reciprocal(rt[:], rt[:])
                drec = dscr.tile([1, 1024], F32, tag="drec", name="drec_t")
                nc.gpsimd.dma_start(drec[:], rt[:])
                for j in range(2):
                    rec = norm.tile([DH, 512], F32, tag="recip", name="recip_t")
                    nc.gpsimd.dma_start(
                        rec[:],
                        drec[:, j * 512 : (j + 1) * 512].to_broadcast((DH, 512)),
                    )
                    if j == 0:
                        nc.vector.tensor_mul(at_t[0:DH, :], accs[j][0:DH, :], rec[:])
                    else:
                        tmp = norm.tile([DH, 512], F32R, tag="normtmp", name="normtmp_t")
                        nc.vector.tensor_mul(tmp[:], accs[j][0:DH, :], rec[:])
                        # engines cannot shift partitions; DMA moves the odd
                        # head's rows into partitions 64-127
                        nc.gpsimd.dma_start(at_t[DH:P, :], tmp[:])
            for sub in range(4):
                o = ps_out.tile([P, DOUT], F32, tag="oproj", name="oproj_acc")
                for p in range(2):
                    nc.tensor.matmul(
                        o[:],
                        attnT[p][:, sub * P : (sub + 1) * P],
                        wo_sb[:, p, :],
                        start=(p == 0),
                        stop=(p == 1),
                    )
                ostage = norm.tile([P, DOUT], F32, tag="ostage", name="ostage_t")
                nc.vector.tensor_copy(ostage[:], o[:])
                row = qt * 512 + sub * P
                nc.gpsimd.dma_start(out[row : row + P, :], ostage[:])
        ps_out_cm.__exit__(None, None, None)
        ps_attn_cm.__exit__(None, None, None)
        ps_scores_cm.__exit__(None, None, None)


def _build():
    nc = bacc.Bacc(
        "TRN2", target_bir_lowering=False, debug=False, enable_asserts=False
    )
    xT = nc.dram_tensor("xT", [DX, NQ], F32, kind="ExternalInput").ap()
    ctxT = nc.dram_tensor("ctxT", [DC, NC], F32, kind="ExternalInput").ap()
    wq = nc.dram_tensor("wq", [DX, COLS], F32, kind="ExternalInput").ap()
    wk = nc.dram_tensor("wk", [DC, COLS], F32, kind="ExternalInput").ap()
    wv = nc.dram_tensor("wv", [DC, COLS], F32, kind="ExternalInput").ap()
    wo = nc.dram_tensor("wo", [COLS, DOUT], F32, kind="ExternalInput").ap()
    ones = nc.dram_tensor("ones", [1, KC * HPC], F32, kind="ExternalInput").ap()
    out = nc.dram_tensor("out", [NQ, DOUT], F32, kind="ExternalOutput").ap()
    with tile.TileContext(nc) as tc:
        _emit(tc, nc, xT, ctxT, wq, wk, wv, wo, ones, out)
    nc.compile()
    return nc


_NC = None


def _get_nc():
    global _NC
    if _NC is None:
        _NC = _build()
    return _NC


def _in_maps(x, context, Wq, Wk, Wv, Wo):
    maps = []
    for c in range(8):
        b, g = c // 2, c % 2
        cs = slice(g * COLS, (g + 1) * COLS)
        maps.append(
            {
                "xT": np.ascontiguousarray(x[b].T),
                "ctxT": np.ascontiguousarray(context[b].T),
                "wq": np.ascontiguousarray(Wq[:, cs]),
                "wk": np.ascontiguousarray(Wk[:, cs]),
                "wv": np.ascontiguousarray(Wv[:, cs]),
                "wo": np.ascontiguousarray(Wo[cs, :]),
                "ones": np.ones((1, KC * HPC), np.float32),
            }
        )
    return maps


def _execute(in_maps, **kw):
    return bass_utils.run_bass_kernel_spmd(
        _get_nc(), in_maps, core_ids=list(range(8)), **kw
    )


def kernel(x, context, Wq, Wk, Wv, Wo, bo):
    x = np.asarray(x, np.float32)
    context = np.asarray(context, np.float32)
    Wq = np.asarray(Wq, np.float32)
    Wk = np.asarray(Wk, np.float32)
    Wv = np.asarray(Wv, np.float32)
    Wo = np.asarray(Wo, np.float32)
    bo = np.asarray(bo, np.float32)
    res = _execute(_in_maps(x, context, Wq, Wk, Wv, Wo))
    parts = [r["out"] for r in res.results]
    out = np.empty((B, NQ, DOUT), np.float32)
    for b in range(B):
        out[b] = parts[2 * b] + parts[2 * b + 1] + bo[None, :]
    return out
